# revision 9
# baseline (speedup 1.0000x reference)
"""Conformer MHSA block on 8 Trainium2 NeuronCores (Bass/Tile).

Data-parallel across the batch: each of the 8 cores processes 2 of the 16
batch rows end to end (LayerNorm -> QKV -> 8-head attention with padding
masks -> output projection -> residual). No collectives.

Layout strategy per core (per batch row b, T=1024 tokens, D=512):
  - LayerNorm runs token-major ([128 tok, 512]); scale/bias are folded into
    the projection weights on the host, so the kernel only standardizes.
  - y is transposed on the PE (128x128 blocks) to yT [d, tok], which feeds
    qT/kT (weights stationary) and v (yT stationary) projections.
  - Attention computes logits TRANSPOSED ([tk, tq]) so softmax's sum runs
    through the matmul: v is stored as vplus [tok, 8, 65] with a ones
    column per head, making the ctx matmul emit the softmax denominator as
    psum row 64. Key-padding masks are applied as per-partition biases in
    the exp; padded queries are zeroed via validq/rowsum and patched with a
    rank-1 (mean over all v) @ wo correction in the output projection.
  - All matmuls run float32r (full PE rate at N=512); absolute accuracy is
    ~2e-4 relative to output scale.
"""
import numpy as np

B, T, D = 16, 1024, 512
H, HD = 8, 64
NB = 2            # batch rows per core
NCORES = 8
R_SOFTPLUS_0 = 1.442695041
LN_EPS = 1e-6
BIG_NEG = -30000.0

_PROGRAM = None


def _build_program(debug=False):
    import sys
    if "/opt/trn_rl_repo" not in sys.path:
        sys.path.insert(0, "/opt/trn_rl_repo")
    import concourse.bass as bass
    import concourse.bacc as bacc
    import concourse.tile as tile
    from concourse import mybir
    from concourse.masks import make_identity

    f32 = mybir.dt.float32
    f32r = mybir.dt.float32r
    AF = mybir.ActivationFunctionType
    ALU = mybir.AluOpType

    nc = bacc.Bacc()

    xs = nc.dram_tensor("xs", [NB, T, D], f32, kind="ExternalInput")
    xp = nc.dram_tensor("xp", [NB, T], f32, kind="ExternalInput")
    wq_d = nc.dram_tensor("wq", [D, D], f32r, kind="ExternalInput")
    wk_d = nc.dram_tensor("wk", [D, D], f32r, kind="ExternalInput")
    wv_d = nc.dram_tensor("wv", [D, D], f32r, kind="ExternalInput")
    wo_d = nc.dram_tensor("wo", [D, D], f32r, kind="ExternalInput")
    bq_d = nc.dram_tensor("bq", [D], f32, kind="ExternalInput")
    bk_d = nc.dram_tensor("bk", [D], f32, kind="ExternalInput")
    bv_d = nc.dram_tensor("bv", [D], f32r, kind="ExternalInput")
    bo_d = nc.dram_tensor("bo", [D], f32r, kind="ExternalInput")
    out_d = nc.dram_tensor("out", [NB, T, D], f32, kind="ExternalOutput")
    rs_scr = nc.dram_tensor("rs_scr", [NB, 4, 2, T], f32)
    dbg = {}
    if debug:
        for nm, shp in (("d_yT0", [128, T]), ("d_qT0", [128, T]), ("d_kT0", [128, T]),
                        ("d_vp0", [128, 520]), ("d_ctxu0", [128, T]),
                        ("d_rs0", [128, T]), ("d_rp0", [128, T]),
                        ("d_kb", [128, 8]), ("d_vqb", [128, T]),
                        ("d_vmean", [128, 4]), ("d_wvm", [1, 512]),
                        ("d_ivq", [1, T])):
            dbg[nm] = nc.dram_tensor(nm, shp, f32, kind="ExternalOutput")

    with tile.TileContext(nc) as tc:
        with (
            tc.tile_pool(name="pers", bufs=1) as pers,
            tc.tile_pool(name="perb", bufs=1) as perb,
            tc.tile_pool(name="stream", bufs=5) as stream,
            tc.tile_pool(name="stats", bufs=8) as stats,
            tc.tile_pool(name="pexp", bufs=4) as pexp,
            tc.tile_pool(name="outp", bufs=3) as outp,
            tc.tile_pool(name="rsp", bufs=1) as rsp,
            tc.tile_pool(name="rpp", bufs=2) as rpp,
            tc.tile_pool(name="ps_big", bufs=2, space="PSUM") as ps_big,
            tc.tile_pool(name="ps_lg", bufs=2, space="PSUM") as ps_lg,
            tc.tile_pool(name="ps_ctx", bufs=4, space="PSUM") as ps_ctx,
        ):
            # ---------------- persistent setup ----------------
            ident = pers.tile([128, 128], f32, tag="ident")
            make_identity(nc, ident)
            ones_f32 = pers.tile([128, 8], f32, tag="ones_f32")
            nc.vector.memset(ones_f32, 1.0)
            eps_t = pers.tile([128, 1], f32, tag="eps")
            nc.vector.memset(eps_t, LN_EPS)
            ones_row = pers.tile([1, 128], f32r, tag="ones_row")
            nc.vector.tensor_copy(ones_row, ones_f32[0:1, 0:1].to_broadcast((1, 128)))
            ones_col = pers.tile([128, 2], f32r, tag="ones_col")
            nc.vector.tensor_copy(ones_col, ones_f32[:, 0:2])

            wq_sb, wk_sb, wv_sb, wo_sb = [], [], [], []
            for c in range(4):
                for (lst, dram, nm) in ((wq_sb, wq_d, "wq"), (wk_sb, wk_d, "wk"),
                                        (wv_sb, wv_d, "wv"), (wo_sb, wo_d, "wo")):
                    t_ = pers.tile([128, 512], f32r, tag=f"{nm}{c}")
                    nc.sync.dma_start(out=t_, in_=dram[c * 128:(c + 1) * 128, :])
                    lst.append(t_)
            bq_sb = pers.tile([128, 4], f32, tag="bq")
            nc.sync.dma_start(out=bq_sb, in_=bq_d.rearrange("(c p) -> p c", p=128))
            bk_sb = pers.tile([128, 4], f32, tag="bk")
            nc.sync.dma_start(out=bk_sb, in_=bk_d.rearrange("(c p) -> p c", p=128))
            bv_row = pers.tile([1, 512], f32r, tag="bv")
            nc.sync.dma_start(out=bv_row, in_=bv_d[:])
            bo_row = pers.tile([1, 512], f32r, tag="bo")
            nc.sync.dma_start(out=bo_row, in_=bo_d[:])

            for b in range(NB):
                # ---------------- per-b mask rows ----------------
                kb_sb = perb.tile([128, 8], f32, tag="kb")
                nc.sync.dma_start(out=kb_sb, in_=xp[b, :].rearrange("(t p) -> p t", p=128))
                nc.scalar.activation(kb_sb, kb_sb, AF.Copy, scale=BIG_NEG)

                vq_row = perb.tile([1, T], f32, tag="vq")
                nc.sync.dma_start(out=vq_row, in_=xp[b, :])
                ivq_row = perb.tile([1, T], f32r, tag="ivq")
                nc.vector.tensor_copy(ivq_row, vq_row)          # = x_paddings (1 at pad)
                nc.scalar.activation(vq_row, vq_row, AF.Identity, bias=1.0, scale=-1.0)
                vq_bcast = perb.tile([128, T], f32, tag="vqb")
                nc.gpsimd.partition_broadcast(vq_bcast, vq_row)

                # ---------------- LN + transpose to yT ----------------
                yT = [perb.tile([128, T], f32r, tag=f"yT{c}", name=f"yT{c}") for c in range(4)]
                for g in range(2):
                    ys = []
                    for t4 in range(4):
                        t = g * 4 + t4
                        x_t = stream.tile([128, 512], f32, tag="x")
                        nc.sync.dma_start(out=x_t, in_=xs[b, t * 128:(t + 1) * 128, :])
                        st6 = stats.tile([128, 6], f32, tag="st6")
                        nc.vector.bn_stats(out=st6, in_=x_t)
                        mv = stats.tile([128, 2], f32, tag="mv")
                        nc.vector.bn_aggr(out=mv, in_=st6)
                        sd = stats.tile([128, 1], f32, tag="sd")
                        nc.scalar.activation(sd, mv[:, 1:2], AF.Sqrt, bias=eps_t)
                        rstd = stats.tile([128, 1], f32, tag="rstd")
                        nc.vector.reciprocal(rstd, sd)
                        y_t = stream.tile([128, 512], f32, tag="y")
                        nc.vector.tensor_scalar(y_t, x_t, mv[:, 0:1], rstd,
                                                ALU.subtract, ALU.mult)
                        ys.append(y_t)
                    for c in range(4):
                        ps_t = ps_big.tile([128, 512], f32, tag="big")
                        for t4 in range(4):
                            nc.tensor.transpose(
                                ps_t[:, t4 * 128:(t4 + 1) * 128],
                                ys[t4][:, c * 128:(c + 1) * 128], ident)
                        nc.vector.tensor_copy(yT[c][:, g * 512:(g + 1) * 512], ps_t)

                # ---------------- qT / kT projections ----------------
                qT = [perb.tile([128, T], f32r, tag=f"qT{c}", name=f"qT{c}") for c in range(4)]
                kT = [perb.tile([128, T], f32r, tag=f"kT{c}", name=f"kT{c}") for c in range(4)]
                for dt_ in range(4):
                    for ch in range(2):
                        sl = slice(ch * 512, (ch + 1) * 512)
                        ps_q = ps_big.tile([128, 512], f32, tag="big")
                        for c in range(4):
                            nc.tensor.matmul(ps_q, wq_sb[c][:, dt_ * 128:(dt_ + 1) * 128],
                                             yT[c][:, sl], start=(c == 0), stop=(c == 3))
                        nc.scalar.activation(qT[dt_][:, sl], ps_q, AF.Identity,
                                             bias=bq_sb[:, dt_:dt_ + 1])
                        ps_k = ps_big.tile([128, 512], f32, tag="big")
                        for c in range(4):
                            nc.tensor.matmul(ps_k, wk_sb[c][:, dt_ * 128:(dt_ + 1) * 128],
                                             yT[c][:, sl], start=(c == 0), stop=(c == 3))
                        nc.scalar.activation(kT[dt_][:, sl], ps_k, AF.Identity,
                                             bias=bk_sb[:, dt_:dt_ + 1])

                # ---------------- v projection -> vplus ----------------
                vplus = [perb.tile([128, 8, 65], f32r, tag=f"vp{t}", name=f"vp{t}") for t in range(8)]
                for tt in range(8):
                    ps_v = ps_big.tile([128, 512], f32, tag="big")
                    for c in range(4):
                        nc.tensor.matmul(ps_v, yT[c][:, tt * 128:(tt + 1) * 128],
                                         wv_sb[c], start=(c == 0), stop=False)
                    nc.tensor.matmul(ps_v, ones_row, bv_row, start=False, stop=True)
                    nc.vector.tensor_copy(
                        vplus[tt][:, :, 0:64],
                        ps_v[:, :].rearrange("p (h e) -> p h e", h=8))
                    nc.vector.tensor_copy(
                        vplus[tt][:, :, 64:65],
                        ones_f32[:, 0:8].rearrange("p (h e) -> p h e", h=8))

                # ---------------- attention ----------------
                ctxu = [perb.tile([128, T], f32r, tag=f"cx{c}", name=f"cx{c}") for c in range(4)]
                for cp in range(4):
                    rs_a = rsp.tile([1, T], f32, tag="rsa")
                    rs_b = rsp.tile([1, T], f32, tag="rsb")
                    for ch in range(2):
                        sl = slice(ch * 512, (ch + 1) * 512)
                        ps_c0 = ps_ctx.tile([65, 512], f32, tag="ctx")
                        ps_c1 = ps_ctx.tile([65, 512], f32, tag="ctx")
                        for tk in range(8):
                            tks = slice(tk * 128, (tk + 1) * 128)
                            ps_l0 = ps_lg.tile([128, 512], f32, tag="lg")
                            nc.tensor.matmul(ps_l0, kT[cp][0:64, tks], qT[cp][0:64, sl],
                                             start=True, stop=True, tile_position=(0, 0))
                            ps_l1 = ps_lg.tile([128, 512], f32, tag="lg")
                            nc.tensor.matmul(ps_l1, kT[cp][64:128, tks], qT[cp][64:128, sl],
                                             start=True, stop=True, tile_position=(64, 0))
                            p0 = pexp.tile([128, 512], f32r, tag="p0")
                            nc.scalar.activation(p0, ps_l0, AF.Exp,
                                                 bias=kb_sb[:, tk:tk + 1])
                            p1 = pexp.tile([128, 512], f32r, tag="p1")
                            nc.scalar.activation(p1, ps_l1, AF.Exp,
                                                 bias=kb_sb[:, tk:tk + 1])
                            nc.tensor.matmul(ps_c0, vplus[tk][:, 2 * cp, 0:65], p0,
                                             start=(tk == 0), stop=(tk == 7))
                            nc.tensor.matmul(ps_c1, vplus[tk][:, 2 * cp + 1, 0:65], p1,
                                             start=(tk == 0), stop=(tk == 7))
                        # extract: unnormalized ctx + softmax denominators
                        nc.vector.tensor_copy(ctxu[cp][0:64, sl], ps_c0[0:64, :])
                        nc.vector.tensor_copy(ctxu[cp][64:128, sl], ps_c1[0:64, :])
                        nc.vector.tensor_copy(rs_a[0:1, sl], ps_c0[64:65, :])
                        nc.vector.tensor_copy(rs_b[0:1, sl], ps_c1[64:65, :])
                    # r'' = validq / rowsum: DRAM-bounce broadcast per head
                    nc.sync.dma_start(out=rs_scr[b, cp, 0, :], in_=rs_a)
                    nc.sync.dma_start(out=rs_scr[b, cp, 1, :], in_=rs_b)
                    rp_t = rpp.tile([128, T], f32, tag="rp")
                    for hh in range(2):
                        row = rs_scr[b, cp, hh, :]
                        row_b = bass.AP(tensor=row.tensor, offset=row.offset,
                                        ap=[[0, 64]] + list(row.ap))
                        nc.sync.dma_start(out=rp_t[hh * 64:(hh + 1) * 64, :], in_=row_b)
                    nc.vector.reciprocal(rp_t, rp_t)
                    nc.vector.tensor_mul(rp_t, rp_t, vq_bcast)
                    if debug and b == 0 and cp == 0:
                        nc.sync.dma_start(out=dbg["d_rs0"][0:1, :], in_=rs_a)
                        nc.sync.dma_start(out=dbg["d_rs0"][64:65, :], in_=rs_b)
                        nc.sync.dma_start(out=dbg["d_rp0"][:, :], in_=rp_t)
                    nc.vector.tensor_mul(ctxu[cp], ctxu[cp], rp_t)

                if debug and b == 0:
                    nc.sync.dma_start(out=dbg["d_yT0"][:, :], in_=yT[0].bitcast(f32))
                    nc.sync.dma_start(out=dbg["d_qT0"][:, :], in_=qT[0].bitcast(f32))
                    nc.sync.dma_start(out=dbg["d_kT0"][:, :], in_=kT[0].bitcast(f32))
                    nc.sync.dma_start(out=dbg["d_vp0"][:, :],
                                      in_=vplus[0].bitcast(f32).rearrange("p h e -> p (h e)"))
                    nc.sync.dma_start(out=dbg["d_ctxu0"][:, :], in_=ctxu[0].bitcast(f32))
                    nc.sync.dma_start(out=dbg["d_kb"][:, :], in_=kb_sb)
                    nc.sync.dma_start(out=dbg["d_vqb"][:, :], in_=vq_bcast)
                    nc.sync.dma_start(out=dbg["d_ivq"][:, :], in_=ivq_row.bitcast(f32))

                # ---------------- vmean @ wo (padded-query correction) ----------------
                vmean_sb = perb.tile([128, 4], f32r, tag="vmean")
                for c in range(4):
                    ps_vma = ps_big.tile([128, 512], f32, tag="big")
                    ps_vmb = ps_big.tile([128, 512], f32, tag="big")
                    for tt in range(8):
                        nc.tensor.matmul(ps_vma[0:64, 0:2],
                                         vplus[tt][:, 2 * c, 0:64],
                                         ones_col, start=(tt == 0), stop=(tt == 7))
                        nc.tensor.matmul(ps_vmb[0:64, 0:2],
                                         vplus[tt][:, 2 * c + 1, 0:64],
                                         ones_col, start=(tt == 0), stop=(tt == 7))
                    nc.scalar.activation(vmean_sb[0:64, c:c + 1], ps_vma[0:64, 0:1],
                                         AF.Copy, scale=1.0 / T)
                    nc.scalar.activation(vmean_sb[64:128, c:c + 1], ps_vmb[0:64, 0:1],
                                         AF.Copy, scale=1.0 / T)
                wvm_row = perb.tile([1, 512], f32r, tag="wvm")
                ps_wv = ps_big.tile([128, 512], f32, tag="big")
                for c in range(4):
                    nc.tensor.matmul(ps_wv[0:1, :], vmean_sb[:, c:c + 1], wo_sb[c],
                                     start=(c == 0), stop=(c == 3))
                nc.scalar.activation(wvm_row, ps_wv[0:1, :], AF.Copy)

                if debug and b == 0:
                    nc.sync.dma_start(out=dbg["d_vmean"][:, :], in_=vmean_sb.bitcast(f32))
                    nc.sync.dma_start(out=dbg["d_wvm"][:, :], in_=wvm_row.bitcast(f32))

                # ---------------- output projection + residual ----------------
                for tt in range(8):
                    tts = slice(tt * 128, (tt + 1) * 128)
                    ps_o = ps_big.tile([128, 512], f32, tag="big")
                    for c in range(4):
                        nc.tensor.matmul(ps_o, ctxu[c][:, tts], wo_sb[c],
                                         start=(c == 0), stop=False)
                    nc.tensor.matmul(ps_o, ones_row, bo_row, start=False, stop=False)
                    nc.tensor.matmul(ps_o, ivq_row[:, tts], wvm_row,
                                     start=False, stop=True)
                    xr = stream.tile([128, 512], f32, tag="xr")
                    nc.sync.dma_start(out=xr, in_=xs[b, tts, :])
                    o_sb = outp.tile([128, 512], f32, tag="o")
                    nc.vector.tensor_add(o_sb, ps_o, xr)
                    nc.sync.dma_start(out=out_d[b, tts, :], in_=o_sb)

    nc.compile()
    return nc


def _fold_weights(inputs):
    lns = inputs["ln_scale"].astype(np.float64)
    lnb = inputs["ln_bias"].astype(np.float64)
    wq = inputs["wq"].reshape(D, D).astype(np.float64)
    wk = inputs["wk"].reshape(D, D).astype(np.float64)
    wv = inputs["wv"].reshape(D, D).astype(np.float64)
    bq = inputs["bq"].reshape(D).astype(np.float64)
    bk = inputs["bk"].reshape(D).astype(np.float64)
    bv = inputs["bv"].reshape(D).astype(np.float64)
    qs = inputs["query_scale"].astype(np.float64)

    sp = np.log1p(np.exp(-np.abs(qs))) + np.maximum(qs, 0)
    qsc = R_SOFTPLUS_0 * sp / np.sqrt(HD)
    qsc_full = np.tile(qsc, H)

    return {
        "wq": np.ascontiguousarray((wq * lns[:, None] * qsc_full[None, :]).astype(np.float32)),
        "bq": np.ascontiguousarray(((bq + lnb @ wq) * qsc_full).astype(np.float32)),
        "wk": np.ascontiguousarray((wk * lns[:, None]).astype(np.float32)),
        "bk": np.ascontiguousarray((bk + lnb @ wk).astype(np.float32)),
        "wv": np.ascontiguousarray((wv * lns[:, None]).astype(np.float32)),
        "bv": np.ascontiguousarray((bv + lnb @ wv).astype(np.float32)),
        "wo": np.ascontiguousarray(inputs["wo"].reshape(D, D).astype(np.float32)),
        "bo": np.ascontiguousarray(inputs["bo"].astype(np.float32)),
    }


def kernel(**inputs):
    global _PROGRAM
    import sys
    if "/opt/trn_rl_repo" not in sys.path:
        sys.path.insert(0, "/opt/trn_rl_repo")
    from concourse.bass_utils import run_bass_kernel_spmd

    if _PROGRAM is None:
        _PROGRAM = _build_program()
    nc = _PROGRAM

    w = _fold_weights(inputs)
    x = np.ascontiguousarray(inputs["x"].astype(np.float32))
    xp_ = np.ascontiguousarray(inputs["x_paddings"].astype(np.float32))

    in_maps = []
    for i in range(NCORES):
        m = {"xs": np.ascontiguousarray(x[i * NB:(i + 1) * NB]),
             "xp": np.ascontiguousarray(xp_[i * NB:(i + 1) * NB])}
        m.update(w)
        in_maps.append(m)

    res = run_bass_kernel_spmd(nc, in_maps, core_ids=list(range(NCORES)))
    return np.concatenate([res.results[i]["out"] for i in range(NCORES)], axis=0)
